# revision 1
# baseline (speedup 1.0000x reference)
"""Attentional pooling layer on Trainium2 (Bass/Tile), 8-core batch-parallel.

Reference computation per batch b:
    scores[hw, n] = sum_c f[c, hw] * w[c, n]          (mm1, fp32)
    num           = softplus(scores)                  (ACT: Abs/Exp/Ln)
    denom[n]      = sum_hw num[hw, n] + 16*CONST      (PE reduce + DVE)
    att[hw, n]    = (num + CONST) / denom[n]          (PE bcast + DVE)
    out[c, n]     = sum_hw f[c, hw] * att[hw, n]      (mm2, float32r)

Partition layout: 3 batches are packed into one 96-partition group at
32-partition offsets (PE tile_position only supports 32-aligned output
partition bases 0/32/64 for small-M matmuls).  mm1 runs M=32 with
zero-padded feature columns so the 16 garbage rows per 32-block are written
with clean zeros.  Partition-dim reductions (sum over hw) and broadcasts
(denom over hw) are done with tiny constant 0/1 matmuls (bd / exp3) fed
from host numpy.  The col-0 matmuls (denominator reduce, broadcast, mm2)
run as float32r (TF32, full PE rate); their operands are rounded to f32r by
the producing ACT/DVE ops.  mm1 stays fp32 (f32r cannot write PSUM at a
nonzero partition base).

32 batches per core = 10 groups of 3 + one ragged group [30, 31, 30] where
the duplicated slot's output is skipped.
"""

import numpy as np
from contextlib import ExitStack

import concourse.bass as bass
import concourse.bacc as bacc
import concourse.tile as tile
from concourse import mybir
from concourse.bass_utils import run_bass_kernel_spmd

F32 = mybir.dt.float32
F32R = mybir.dt.float32r
AF = mybir.ActivationFunctionType
ALU = mybir.AluOpType

N_CORES = 8
B_FULL, C, H, W, N = 256, 256, 4, 4, 2048
HW = H * W                  # 16
B = B_FULL // N_CORES       # 32 batches per core
KC = C // 128               # 2 contraction chunks of 128
GB = 3                      # batches per partition group (32-part offsets 0/32/64)
GP = 32 * GB                # 96 partitions used per group
NCH = 4                     # n chunks per group chain
NW = N // NCH               # 512 (one PSUM bank)
CONST = 1e-4


def make_groups(n_batch):
    """Chunks of GB batches; ragged tail padded with duplicates (emit=False)."""
    groups = []
    for s in range(0, n_batch, GB):
        real = list(range(s, min(s + GB, n_batch)))
        emit = [True] * len(real)
        while len(real) < GB:
            real.append(real[0])
            emit.append(False)
        groups.append((real, emit))
    return groups


def aux_inputs():
    # bd[k, m] = 1 iff row k is one of batch-slot m's real hw rows
    bd = np.zeros((GP, GB), np.float32)
    for k in range(GP):
        if k % 32 < HW:
            bd[k, k // 32] = 1.0
    # exp3[k, m] = 1 iff partition m belongs to batch-slot k's 32-block
    exp3 = np.zeros((GB, GP), np.float32)
    for m in range(GP):
        exp3[m // 32, m] = 1.0
    iden = np.eye(128, dtype=np.float32)
    return {"bd": bd, "exp3": exp3, "iden": iden}


def build_nc(n_batch=B, debug=False):
    nc = bacc.Bacc(None, target_bir_lowering=False, debug=debug)
    feat = nc.dram_tensor("fpad", [128, KC, n_batch, 32], F32, kind="ExternalInput")
    wts = nc.dram_tensor("weights", [n_batch, C, N], F32, kind="ExternalInput")
    out = nc.dram_tensor("out", [n_batch, C, N], F32, kind="ExternalOutput")
    bd_d = nc.dram_tensor("bd", [GP, GB], F32R, kind="ExternalInput")
    exp_d = nc.dram_tensor("exp3", [GB, GP], F32R, kind="ExternalInput")
    id_d = nc.dram_tensor("iden", [128, 128], F32, kind="ExternalInput")

    # [ci, b, kc, n] views of the DRAM tensors
    wts_r = wts.ap().rearrange("b (kc ci) n -> ci b kc n", kc=KC)
    out_r = out.ap().rearrange("b (kc ci) n -> ci b kc n", kc=KC)

    with tile.TileContext(nc) as tc, ExitStack() as ctx:
        singles = ctx.enter_context(tc.tile_pool(name="singles", bufs=1))
        wpool = ctx.enter_context(tc.tile_pool(name="w", bufs=5))
        opool = ctx.enter_context(tc.tile_pool(name="o", bufs=3))
        numpool = ctx.enter_context(tc.tile_pool(name="num", bufs=3))
        attpool = ctx.enter_context(tc.tile_pool(name="att", bufs=2))
        smallpool = ctx.enter_context(tc.tile_pool(name="small", bufs=3))
        ftpool = ctx.enter_context(tc.tile_pool(name="ft", bufs=2))
        ps_sc = ctx.enter_context(tc.tile_pool(name="ps_sc", bufs=4, space="PSUM"))
        ps_dr = ctx.enter_context(tc.tile_pool(name="ps_dr", bufs=1, space="PSUM"))
        ps_ft = ctx.enter_context(tc.tile_pool(name="ps_ft", bufs=1, space="PSUM"))
        ps_o = ctx.enter_context(tc.tile_pool(name="ps_o", bufs=2, space="PSUM"))

        bd_t = singles.tile([GP, GB], F32R)
        nc.sync.dma_start(out=bd_t, in_=bd_d.ap())
        exp_t = singles.tile([GB, GP], F32R)
        nc.sync.dma_start(out=exp_t, in_=exp_d.ap())
        id_t = singles.tile([128, 128], F32)
        nc.sync.dma_start(out=id_t, in_=id_d.ap())

        # features, pre-transposed + hw-padded to 32 with zeros on the host
        f_t = singles.tile([128, KC, n_batch, 32], F32)
        nc.sync.dma_start(out=f_t, in_=feat.ap())

        ev = 0
        for bs, emit in make_groups(n_batch):
            w_t = {}
            for b in set(bs):
                w_t[b] = wpool.tile([128, KC, N], F32, tag="w", name="w_t")
                nc.sync.dma_start(out=w_t[b], in_=wts_r[:, b])

            # transposed features fT[hw, c] for mm2.  Transposing the full
            # zero-padded [128, nreal, 32] slice puts slot j's fT at
            # partition 32j (transpose outputs must start at partition 0).
            nreal = len(set(bs))
            ft_ps = ps_ft.tile([32 * nreal, KC, 128], F32, name="ft_ps")
            for kc in range(KC):
                nc.tensor.transpose(
                    ft_ps[:, kc, :],
                    f_t[:, kc, bs[0] : bs[0] + nreal, :],
                    id_t,
                )
            ft_sb = ftpool.tile([32 * nreal, KC, 128], F32R, name="ft_sb")
            nc.scalar.copy(ft_sb, ft_ps)

            att_t = attpool.tile([GP, NCH, NW], F32R)
            # mm1 for all chunks first, then phase the ACT work (all Abs+Exp,
            # then all Lns) so the table-set switches happen twice per group
            # instead of twice per chunk; explicit deps pin the ACT order.
            sc_l, te_l, tl_l = [], [], []
            for nb in range(NCH):
                sc_ps = ps_sc.tile([GP, NW], F32, name="sc_ps")
                for j in range(GB):
                    for kc in range(KC):
                        nc.tensor.matmul(
                            sc_ps[32 * j : 32 * j + 32, :],
                            f_t[:, kc, bs[j], :],
                            w_t[bs[j]][:, kc, nb * NW : (nb + 1) * NW],
                            start=(kc == 0),
                            stop=(kc == KC - 1),
                        )
                sc_l.append(sc_ps)
            # softplus(x) = max(x,0) + ln(1 + exp(-|x|)): exp arg <= 0 so no
            # overflow, Ln input stays in [1,2]
            exp_insts = []
            for nb in range(NCH):
                t_abs = numpool.tile([GP, NW], F32, tag="tabs")
                nc.scalar.activation(t_abs, sc_l[nb], AF.Abs)
                t_exp = numpool.tile([GP, NW], F32, tag="texp", bufs=NCH)
                exp_insts.append(
                    nc.scalar.activation(t_exp, t_abs, AF.Exp, scale=-1.0)
                )
                te_l.append(t_exp)
            for nb in range(NCH):
                t_ln = numpool.tile([GP, NW], F32, tag="tln", bufs=NCH)
                ln_i = nc.scalar.activation(t_ln, te_l[nb], AF.Ln, bias=1.0)
                tile.add_dep_helper(
                    ln_i.ins, exp_insts[-1].ins, sync=False,
                    reason="cluster Lns after all Exps (one table switch)",
                )
                tl_l.append(t_ln)
            for nb in range(NCH):
                num_t = numpool.tile([GP, NW], F32R, tag="num")
                nc.vector.scalar_tensor_tensor(
                    num_t, sc_l[nb], 0.0, tl_l[nb], op0=ALU.max, op1=ALU.add
                )
                d_ps = ps_dr.tile([GB, NW], F32, tag="dr", name="d_ps")
                nc.tensor.matmul(
                    d_ps,
                    bd_t,
                    num_t,
                    start=True,
                    stop=True,
                )
                r_t = smallpool.tile([GB, NW], F32R)
                with nc.allow_low_precision(reason="tf32 matmul operand"):
                    nc.vector.tensor_scalar_add(r_t, d_ps, HW * CONST)
                    nc.vector.reciprocal(r_t, r_t)
                rb_ps = ps_dr.tile([GP, NW], F32, tag="dr", name="rb_ps")
                nc.tensor.matmul(
                    rb_ps,
                    exp_t,
                    r_t,
                    start=True,
                    stop=True,
                )
                # att = (num + CONST) * (1/denom)
                nc.vector.scalar_tensor_tensor(
                    att_t[:, nb, :],
                    num_t,
                    CONST,
                    rb_ps,
                    op0=ALU.add,
                    op1=ALU.mult,
                )

            for j in range(GB):
                if not emit[j]:
                    continue
                o_sb = opool.tile([128, KC, N], F32, tag="o", name="o_sb")
                for kc in range(KC):
                    for nb in range(NCH):
                        o_ps = ps_o.tile([128, NW], F32)
                        nc.tensor.matmul(
                            o_ps,
                            ft_sb[32 * j : 32 * j + HW, kc, :],
                            att_t[32 * j : 32 * j + HW, nb, :],
                            start=True,
                            stop=True,
                        )
                        dst = o_sb[:, kc, nb * NW : (nb + 1) * NW]
                        if ev % 2 == 0:
                            nc.vector.tensor_copy(dst, o_ps)
                        else:
                            nc.scalar.copy(dst, o_ps)
                        ev += 1
                nc.sync.dma_start(out=out_r[:, bs[j]], in_=o_sb)

    nc.compile()
    return nc


_NC_CACHE = {}


def _get_nc(n_batch=B):
    if n_batch not in _NC_CACHE:
        _NC_CACHE[n_batch] = build_nc(n_batch)
    return _NC_CACHE[n_batch]


def prep_features(features, dtype=np.float32):
    """[nb, C, H, W] f32 -> padded [128, KC, nb, 32] in dtype."""
    features = np.asarray(features).astype(dtype)
    nb = features.shape[0]
    f4 = features.reshape(nb, KC, 128, HW)
    fpad = np.zeros((nb, KC, 128, 32), dtype)
    fpad[..., :HW] = f4
    return np.ascontiguousarray(fpad.transpose(2, 1, 0, 3))  # [128, KC, nb, 32]


def run(features, weights, trace=False, **kwargs):
    """Shard over 8 cores, run, gather. Returns (out, BassKernelResults)."""
    fpad = prep_features(features)
    weights = np.ascontiguousarray(np.asarray(weights), dtype=np.float32)
    aux = aux_inputs()
    nc = _get_nc()
    in_maps = []
    for i in range(N_CORES):
        sl = slice(i * B, (i + 1) * B)
        in_maps.append(
            {"fpad": fpad[:, :, sl], "weights": weights[sl], **aux}
        )
    res = run_bass_kernel_spmd(
        nc, in_maps, core_ids=list(range(N_CORES)), trace=trace, **kwargs
    )
    out = np.concatenate([r["out"] for r in res.results], axis=0).astype(np.float32)
    return out, res


def kernel(features, weights):
    out, _ = run(features, weights)
    return out



# revision 10
# speedup vs baseline: 1.6066x; 1.6066x over previous
"""Attentional pooling layer on Trainium2 (Bass/Tile), 8-core batch-parallel.

Reference computation per batch b:
    scores[hw, n] = sum_c f[c, hw] * w[c, n]          (mm1)
    num           = softplus(scores)                  (ACT: Abs/Exp/Ln + DVE)
    denom[n]      = sum_hw num[hw, n] + 16*CONST      (PE reduce + ACT add)
    att[hw, n]    = (num + CONST) / denom[n]          (DVE recip, PE bcast, DVE)
    out[c, n]     = sum_hw f[c, hw] * att[hw, n]      (mm2)

The kernel is DMA-bound (weights in + out out dominate), so both weights and
output travel as bf16 (tolerance is 2e-2; bf16 end-to-end lands ~1e-3).
All matmuls take bf16 moving operands.  4 batches are packed per 128-partition
group at 32-partition offsets 0/32/64/96 (PE tile_position); mm1 runs M=32
with zero-padded feature columns so the 16 garbage rows per 32-block hold
clean zeros.  Partition-dim reduction (sum over hw) and broadcast (denom over
hw) are tiny constant 0/1 matmuls (bd / exp4) fed from host numpy.  The
features transposed for mm2 (fT[hw, c] at each batch slot's partition offset)
are packed host-side and DMA'd once — no PE transposes.

32 batches per core = 8 groups of 4, no ragged tail.
"""

import numpy as np
import ml_dtypes
from contextlib import ExitStack

import concourse.bass as bass
import concourse.bacc as bacc
import concourse.tile as tile
from concourse import mybir
from concourse.bass_utils import run_bass_kernel_spmd

F32 = mybir.dt.float32
BF16 = mybir.dt.bfloat16
NPBF16 = ml_dtypes.bfloat16
AF = mybir.ActivationFunctionType
ALU = mybir.AluOpType

N_CORES = 8
B_FULL, C, H, W, N = 256, 256, 4, 4, 2048
HW = H * W                  # 16
B = B_FULL // N_CORES       # 32 batches per core
KC = C // 128               # 2 contraction chunks of 128
GB = 4                      # batches per partition group (offsets 0/32/64/96)
NG = B // GB                # 8 groups
NCH = 4                     # n chunks per group chain
NW = N // NCH               # 512 (one PSUM bank)
CONST = 1e-4


def aux_inputs():
    # bd[k, m] = 1 iff row k is one of batch-slot m's real hw rows
    bd = np.zeros((128, GB), NPBF16)
    for k in range(128):
        if k % 32 < HW:
            bd[k, k // 32] = 1.0
    # exp4[m, p] = 1 iff partition p belongs to batch-slot m's 32-block
    exp4 = np.zeros((GB, 128), NPBF16)
    for p in range(128):
        exp4[p // 32, p] = 1.0
    c16 = np.full((128, 1), HW * CONST, np.float32)
    return {"bd": bd, "exp4": exp4, "c16": c16}


def build_nc(debug=False):
    nc = bacc.Bacc(None, target_bir_lowering=False, debug=debug)
    feat = nc.dram_tensor("fpad", [128, KC, B, 32], BF16, kind="ExternalInput")
    ftp = nc.dram_tensor("ftpack", [128, NG, GB, KC, 128], BF16, kind="ExternalInput")
    wts = nc.dram_tensor("weights", [B, C, N], BF16, kind="ExternalInput")
    out = nc.dram_tensor("out", [B, C, N], BF16, kind="ExternalOutput")
    bd_d = nc.dram_tensor("bd", [128, GB], BF16, kind="ExternalInput")
    exp_d = nc.dram_tensor("exp4", [GB, 128], BF16, kind="ExternalInput")
    c16_d = nc.dram_tensor("c16", [128, 1], F32, kind="ExternalInput")

    # [ci, b, kc, nch, nw] views of the DRAM tensors
    wts_r = wts.ap().rearrange("b (kc ci) (nch nw) -> ci b kc nch nw", kc=KC, nch=NCH)
    out_r = out.ap().rearrange("b (kc ci) (nch nw) -> ci b kc nch nw", kc=KC, nch=NCH)

    with tile.TileContext(nc) as tc, ExitStack() as ctx:
        singles = ctx.enter_context(tc.tile_pool(name="singles", bufs=1))
        wpool = ctx.enter_context(tc.tile_pool(name="w", bufs=12))
        opool = ctx.enter_context(tc.tile_pool(name="o", bufs=3))
        numpool = ctx.enter_context(tc.tile_pool(name="num", bufs=6))
        attpool = ctx.enter_context(tc.tile_pool(name="att", bufs=2))
        smallpool = ctx.enter_context(tc.tile_pool(name="small", bufs=3))
        ps_sc = ctx.enter_context(tc.tile_pool(name="ps_sc", bufs=2, space="PSUM"))
        ps_d = ctx.enter_context(tc.tile_pool(name="ps_d", bufs=1, space="PSUM"))
        ps_rb = ctx.enter_context(tc.tile_pool(name="ps_rb", bufs=1, space="PSUM"))
        ps_o = ctx.enter_context(tc.tile_pool(name="ps_o", bufs=2, space="PSUM"))

        bd_t = singles.tile([128, GB], BF16)
        nc.sync.dma_start(out=bd_t, in_=bd_d.ap())
        exp_t = singles.tile([GB, 128], BF16)
        nc.sync.dma_start(out=exp_t, in_=exp_d.ap())
        c16_t = singles.tile([128, 1], F32)
        nc.sync.dma_start(out=c16_t, in_=c16_d.ap())

        # features, pre-transposed + hw-padded to 32 with zeros on the host
        f_t = singles.tile([128, KC, B, 32], BF16)
        nc.sync.dma_start(out=f_t, in_=feat.ap())
        # fT[hw, c] per batch at its slot's partition offset, packed host-side
        # with zeros outside the slot (full-K mm2: operand base partition must
        # be 0/32/64, so slot 3 can't be addressed directly at 96)
        ftp_t = singles.tile([128, NG, GB, KC, 128], BF16)
        nc.sync.dma_start(out=ftp_t, in_=ftp.ap())

        ev = 0
        for g in range(NG):
            bs = [GB * g + j for j in range(GB)]
            w_t = {}
            for b in bs:
                w_t[b] = wpool.tile([128, KC, NCH, NW], BF16, tag="w", name="w_t")
                nc.sync.dma_start(out=w_t[b], in_=wts_r[:, b])

            att_t = attpool.tile([128, NCH, NW], BF16)
            for nb in range(NCH):
                sc_ps = ps_sc.tile([128, NW], F32, tag="sc", name="sc_ps")
                for j in range(GB):
                    for kc in range(KC):
                        nc.tensor.matmul(
                            sc_ps[32 * j : 32 * j + 32, :],
                            f_t[:, kc, bs[j], :],
                            w_t[bs[j]][:, kc, nb, :],
                            start=(kc == 0),
                            stop=(kc == KC - 1),
                            tile_position=(0, 32 * j),
                        )
                # softplus(x) = max(x,0) + ln(1 + exp(-|x|)): exp arg <= 0 so
                # no overflow, Ln input stays in [1,2]
                t_abs = numpool.tile([128, NW], F32, tag="tabs")
                nc.scalar.activation(t_abs, sc_ps, AF.Abs)
                t_exp = numpool.tile([128, NW], F32, tag="texp")
                nc.scalar.activation(t_exp, t_abs, AF.Exp, scale=-1.0)
                t_ln = numpool.tile([128, NW], F32, tag="tln")
                nc.scalar.activation(t_ln, t_exp, AF.Ln, bias=1.0)
                num_t = numpool.tile([128, NW], BF16, tag="num")
                with nc.allow_low_precision(reason="bf16 within 2e-2 tolerance"):
                    nc.vector.scalar_tensor_tensor(
                        num_t, sc_ps, 0.0, t_ln, op0=ALU.max, op1=ALU.add
                    )
                d_ps = ps_d.tile([GB, NW], F32, tag="d", name="d_ps")
                nc.tensor.matmul(d_ps, bd_t, num_t, start=True, stop=True)
                rtmp = smallpool.tile([GB, NW], F32, tag="rt")
                # denom = sum_hw softplus + 16*CONST
                nc.scalar.activation(rtmp, d_ps, AF.Identity, bias=c16_t[:GB])
                r_t = smallpool.tile([GB, NW], BF16, tag="rb")
                with nc.allow_low_precision(reason="bf16 within 2e-2 tolerance"):
                    nc.vector.reciprocal(r_t, rtmp)
                rb_ps = ps_rb.tile([128, NW], F32, tag="rb", name="rb_ps")
                nc.tensor.matmul(rb_ps, exp_t, r_t, start=True, stop=True)
                # att = (num + CONST) * (1/denom)
                with nc.allow_low_precision(reason="bf16 within 2e-2 tolerance"):
                    nc.vector.scalar_tensor_tensor(
                        att_t[:, nb, :],
                        num_t,
                        CONST,
                        rb_ps,
                        op0=ALU.add,
                        op1=ALU.mult,
                    )

            for j in range(GB):
                o_sb = opool.tile([128, KC, NCH, NW], BF16, tag="o", name="o_sb")
                for kc in range(KC):
                    for nbp in range(NCH // 2):
                        o_ps = ps_o.tile([128, 2, NW], F32, tag="o", name="o_ps")
                        for h in range(2):
                            nb = 2 * nbp + h
                            nc.tensor.matmul(
                                o_ps[:, h, :],
                                ftp_t[:, g, j, kc, :],
                                att_t[:, nb, :],
                                start=True,
                                stop=True,
                            )
                        dst = o_sb[:, kc, 2 * nbp : 2 * nbp + 2, :]
                        if ev % 2 == 0:
                            nc.scalar.copy(dst, o_ps)
                        else:
                            with nc.allow_low_precision(
                                reason="bf16 within 2e-2 tolerance"
                            ):
                                nc.vector.tensor_copy(dst, o_ps)
                        ev += 1
                nc.sync.dma_start(out=out_r[:, bs[j]], in_=o_sb)

    nc.compile()
    return nc


_NC_CACHE = {}


def _get_nc():
    if "nc" not in _NC_CACHE:
        _NC_CACHE["nc"] = build_nc()
    return _NC_CACHE["nc"]


def prep_features(features):
    """[B_FULL, C, H, W] f32 -> (fpad [128, KC, B_FULL, 32],
    ftpack [128, ngrp_total, GB, KC, 128]) both bf16.

    fpad[ci, kc, b, h] = f[b, kc*128+ci, h] (h < 16; zero-padded to 32).
    ftpack[32j+h, G, j, kc, ci] = f[4G+j, kc*128+ci, h], zero for partitions
    outside slot j's 16 real rows, with G a global group index (batches
    4G..4G+3); per-core slices are taken along G/b later.
    """
    f = np.asarray(features, np.float32).reshape(B_FULL, KC, 128, HW)
    fpad = np.zeros((B_FULL, KC, 128, 32), np.float32)
    fpad[..., :HW] = f
    fpad = np.ascontiguousarray(fpad.transpose(2, 1, 0, 3)).astype(NPBF16)

    ngrp = B_FULL // GB
    fg = f.reshape(ngrp, GB, KC, 128, HW)
    ftp = np.zeros((GB, 32, ngrp, GB, KC, 128), np.float32)
    for j in range(GB):
        # [G, kc, ci, h] -> [h, G, kc, ci]
        ftp[j, :HW, :, j] = fg[:, j].transpose(3, 0, 1, 2)
    ftp = np.ascontiguousarray(ftp.reshape(128, ngrp, GB, KC, 128)).astype(NPBF16)
    return fpad, ftp


def run(features, weights, trace=False, **kwargs):
    """Shard over 8 cores, run, gather. Returns (out, BassKernelResults)."""
    fpad, ftp = prep_features(features)
    weights = np.asarray(weights, np.float32).astype(NPBF16)
    aux = aux_inputs()
    nc = _get_nc()
    in_maps = []
    for i in range(N_CORES):
        sl = slice(i * B, (i + 1) * B)
        gsl = slice(i * NG, (i + 1) * NG)
        in_maps.append(
            {
                "fpad": np.ascontiguousarray(fpad[:, :, sl]),
                "ftpack": np.ascontiguousarray(ftp[:, gsl]),
                "weights": weights[sl],
                **aux,
            }
        )
    res = run_bass_kernel_spmd(
        nc, in_maps, core_ids=list(range(N_CORES)), trace=trace, **kwargs
    )
    out = np.concatenate([r["out"] for r in res.results], axis=0).astype(np.float32)
    return out, res


def kernel(features, weights):
    out, _ = run(features, weights)
    return out


# revision 50
# speedup vs baseline: 2.0253x; 1.2607x over previous
"""Attentional pooling layer on Trainium2 (Bass/Tile), 8-core batch-parallel.

Reference computation per batch b:
    scores[hw, n] = sum_c f[c, hw] * w[c, n]          (mm1)
    num           = softplus(scores)                  (DVE abs/relu + ACT Exp/Ln)
    denom[n]      = sum_hw num[hw, n] + 16*CONST      (PE reduce + ACT add)
    att[hw, n]    = (num + CONST) / denom[n]          (DVE recip, PE bcast, DVE)
    out[c, n]     = sum_hw f[c, hw] * att[hw, n]      (mm2)

The kernel is DMA-bound (weights in + out out dominate), so both weights and
output travel as bf16 (tolerance is 2e-2; bf16 end-to-end lands ~4e-3), and
all matmuls take bf16 moving operands.  4 batches are packed per 128-partition
group at 32-partition offsets 0/32/64/96 (PE tile_position); mm1 runs M=32
with zero-padded feature columns so the 16 garbage rows per 32-block hold
clean zeros.  Partition-dim reduction (sum over hw) and broadcast (denom over
hw) are tiny constant 0/1 matmuls (bd / exp4) fed from host numpy.

The per-group tail (reduce/broadcast/mm2) is software-pipelined one group
behind mm1 so the in-order PE queue never waits on the ACT/DVE softplus
chain: PE program is mm1(g) | denom+bcast(g-1) | mm2(g-1).  ACT table
switches are limited to two per group by phasing all Exps then all Lns
(Identity / Copy live in every table).  mm2 uses fT[hw, c] slices packed
host-side; slot 3 sits at partition 96, which matmul operands cannot address
(base must be 0/32/64), so slot 3 gets a full-K zero-padded stationary tile
instead.  PSUM->SBUF output copies rotate over Pool/ACT/DVE.

32 batches per core = 8 groups of 4, no ragged tail.
"""

import numpy as np
import ml_dtypes
from contextlib import ExitStack

import concourse.bass as bass
import concourse.bacc as bacc
import concourse.tile as tile
from concourse import mybir
from concourse.bass_utils import run_bass_kernel_spmd

F32 = mybir.dt.float32
BF16 = mybir.dt.bfloat16
NPBF16 = ml_dtypes.bfloat16
AF = mybir.ActivationFunctionType
ALU = mybir.AluOpType

N_CORES = 8
B_FULL, C, H, W, N = 256, 256, 4, 4, 2048
HW = H * W                  # 16
B = B_FULL // N_CORES       # 32 batches per core
KC = C // 128               # 2 contraction chunks of 128
GB = 4                      # batches per partition group (offsets 0/32/64/96)
NG = B // GB                # 8 groups
NCH = 4                     # n chunks per group chain
NW = N // NCH               # 512 (one PSUM bank)
CONST = 1e-4


def aux_inputs():
    # bd[k, m] = 1 iff row k is one of batch-slot m's real hw rows
    bd = np.zeros((128, GB), NPBF16)
    for k in range(128):
        if k % 32 < HW:
            bd[k, k // 32] = 1.0
    # exp4[m, p] = 1 iff partition p belongs to batch-slot m's 32-block
    exp4 = np.zeros((GB, 128), NPBF16)
    for p in range(128):
        exp4[p // 32, p] = 1.0
    c16 = np.full((128, 1), HW * CONST, np.float32)
    return {"bd": bd, "exp4": exp4, "c16": c16}


def build_nc(debug=False):
    nc = bacc.Bacc(None, target_bir_lowering=False, debug=debug)
    feat = nc.dram_tensor("fpad", [128, KC, B, 32], BF16, kind="ExternalInput")
    ftp = nc.dram_tensor("ftpack", [128, NG, KC, 128], BF16, kind="ExternalInput")
    ftp3 = nc.dram_tensor("ftpad3", [128, NG, KC, 128], BF16, kind="ExternalInput")
    wts = nc.dram_tensor("weights", [B, C, N], BF16, kind="ExternalInput")
    out = nc.dram_tensor("out", [B, C, N], BF16, kind="ExternalOutput")
    bd_d = nc.dram_tensor("bd", [128, GB], BF16, kind="ExternalInput")
    exp_d = nc.dram_tensor("exp4", [GB, 128], BF16, kind="ExternalInput")
    c16_d = nc.dram_tensor("c16", [128, 1], F32, kind="ExternalInput")

    # [ci, b, kc, nch, nw] views of the DRAM tensors
    wts_r = wts.ap().rearrange("b (kc ci) (nch nw) -> ci b kc nch nw", kc=KC, nch=NCH)
    out_r = out.ap().rearrange("b (kc ci) (nch nw) -> ci b kc nch nw", kc=KC, nch=NCH)

    with tile.TileContext(nc) as tc, ExitStack() as ctx:
        singles = ctx.enter_context(tc.tile_pool(name="singles", bufs=1))
        wpool = ctx.enter_context(tc.tile_pool(name="w", bufs=12))
        opool = ctx.enter_context(tc.tile_pool(name="o", bufs=6))
        numpool = ctx.enter_context(tc.tile_pool(name="num", bufs=6))
        attpool = ctx.enter_context(tc.tile_pool(name="att", bufs=3))
        smallpool = ctx.enter_context(tc.tile_pool(name="small", bufs=3))
        ps_sc = ctx.enter_context(tc.tile_pool(name="ps_sc", bufs=2, space="PSUM"))
        ps_dr = ctx.enter_context(tc.tile_pool(name="ps_dr", bufs=2, space="PSUM"))
        ps_o = ctx.enter_context(tc.tile_pool(name="ps_o", bufs=2, space="PSUM"))

        w_t = {}

        def issue_weights(g):
            for b in range(GB * g, GB * (g + 1)):
                w_t[b] = wpool.tile([128, KC, NCH, NW], BF16, tag="w", name="w_t")
                nc.sync.dma_start(out=w_t[b], in_=wts_r[:, b])

        # features first (mm1's stationary), then the first weight group, then
        # the small/later-needed tensors — keeps the DMA queue dense at start
        # while letting mm1 begin as early as possible
        f_t = singles.tile([128, KC, B, 32], BF16, name="f_t")
        nc.sync.dma_start(out=f_t, in_=feat.ap())
        bd_t = singles.tile([128, GB], BF16, name="bd_t")
        nc.sync.dma_start(out=bd_t, in_=bd_d.ap())
        issue_weights(0)
        exp_t = singles.tile([GB, 128], BF16, name="exp_t")
        nc.sync.dma_start(out=exp_t, in_=exp_d.ap())
        c16_t = singles.tile([128, 1], F32, name="c16_t")
        nc.sync.dma_start(out=c16_t, in_=c16_d.ap())
        # fT[hw, c] per batch at its slot's partition offset (mm2 stationary);
        # slot 3 additionally as a full-K tile with zeros outside rows 96..111
        ftp_t = singles.tile([128, NG, KC, 128], BF16, name="ftp_t")
        nc.sync.dma_start(out=ftp_t, in_=ftp.ap())
        ftp3_t = singles.tile([128, NG, KC, 128], BF16, name="ftp3_t")
        nc.sync.dma_start(out=ftp3_t, in_=ftp3.ap())

        # copy-engine rotation per [128, 2*NW] pair-copy: GPSIMD cannot read
        # PSUM on real TRN2, so the PSUM->SBUF output copies are split across
        # ACT (x9) and DVE (x7) only; Pool instead computes num = relu + ln
        # (all-SBUF) each chunk
        COPY_ENG = ["A", "D", "A", "D", "A", "A", "D", "A",
                    "D", "A", "D", "A", "A", "D", "A", "D"]
        state = {}      # g -> dict with relu_l, ln_l, att_t
        pins = {"last_ln": None}

        def mm1_chunk(g, nb):
            """mm1 for chunk nb of group g + DVE abs/relu freeing sc_ps."""
            st = state[g]
            sc_ps = ps_sc.tile([128, NW], F32, tag="sc", name="sc_ps")
            for j in range(GB):
                for kc in range(KC):
                    nc.tensor.matmul(
                        sc_ps[32 * j : 32 * j + 32, :],
                        f_t[:, kc, GB * g + j, :],
                        w_t[GB * g + j][:, kc, nb, :],
                        start=(kc == 0),
                        stop=(kc == KC - 1),
                        tile_position=(0, 32 * j),
                    )
            # softplus(x) = max(x,0) + ln(1 + exp(-|x|)): exp arg <= 0 so
            # no overflow, Ln input stays in [1,2].  relu/abs on DVE free
            # sc_ps right away and keep ACT on Exp/Ln only.  A DVE op may
            # read PSUM only once, so |x| = 2*relu(x) - x (exact in f32).
            t_relu = numpool.tile([128, NW], F32, tag="trelu", bufs=9, name="t_relu")
            t_abs = numpool.tile([128, NW], BF16, tag="tabs", bufs=6, name="t_abs")
            with nc.allow_low_precision(reason="bf16 within 2e-2 tolerance"):
                nc.vector.tensor_scalar_max(t_relu, sc_ps, 0.0)
                nc.vector.scalar_tensor_tensor(
                    t_abs, t_relu, 2.0, sc_ps, op0=ALU.mult, op1=ALU.subtract
                )
            st["abs_l"].append(t_abs)
            st["relu_l"].append(t_relu)

        def act_phases(g):
            """Exp then Ln for group g, each phase clustered so the ACT
            table loads stay at two per group."""
            st = state[g]
            exp_l, exp_il = [], []
            for nb in range(NCH):
                t_exp = numpool.tile([128, NW], BF16, tag="texp", bufs=6, name="t_exp")
                with nc.allow_low_precision(reason="bf16 within 2e-2 tolerance"):
                    ei = nc.scalar.activation(
                        t_exp, st["abs_l"][nb], AF.Exp, scale=-1.0
                    )
                if pins["last_ln"] is not None:
                    tile.add_dep_helper(
                        ei.ins, pins["last_ln"].ins, sync=False,
                        reason="keep group g Exps after group g-1 Lns",
                    )
                exp_l.append(t_exp)
                exp_il.append(ei)
            for nb in range(NCH):
                t_ln = numpool.tile([128, NW], BF16, tag="tln", bufs=9, name="t_ln")
                with nc.allow_low_precision(reason="bf16 within 2e-2 tolerance"):
                    ln_i = nc.scalar.activation(t_ln, exp_l[nb], AF.Ln, bias=1.0)
                tile.add_dep_helper(
                    ln_i.ins, exp_il[-1].ins, sync=False,
                    reason="cluster Lns after all Exps (one table switch)",
                )
                st["ln_l"].append(t_ln)
                pins["last_ln"] = ln_i

        def num_phase(g):
            """num = relu + ln for group g on DVE (emitted first in the next
            iteration so the denom matmuls never stall the PE queue)."""
            st = state[g]
            st["num_l"] = []
            for nb in range(NCH):
                num_t = numpool.tile([128, NW], BF16, tag="num", bufs=6, name="num_t")
                with nc.allow_low_precision(reason="bf16 within 2e-2 tolerance"):
                    # all-SBUF, so this can run on GPSIMD (which cannot read
                    # PSUM and would otherwise sit idle)
                    nc.gpsimd.tensor_add(num_t, st["relu_l"][nb], st["ln_l"][nb])
                st["num_l"].append(num_t)

        def tail(g):
            """denom/recip/bcast/att for group g."""
            st = state[g]
            att_t = st["att_t"] = attpool.tile([128, NCH, NW], BF16, name="att_t")
            num_l = st["num_l"]
            for nb in range(NCH):
                d_ps = ps_dr.tile([GB, NW], F32, tag="dr", name="d_ps")
                nc.tensor.matmul(d_ps, bd_t, num_l[nb], start=True, stop=True)
                rtmp = smallpool.tile([GB, NW], F32, tag="rt", name="rtmp")
                # denom = sum_hw softplus + 16*CONST (Identity is in every
                # activation table, so this triggers no table load)
                nc.scalar.activation(rtmp, d_ps, AF.Identity, bias=c16_t[:GB])
                r_t = smallpool.tile([GB, NW], BF16, tag="rb", name="r_t")
                with nc.allow_low_precision(reason="bf16 within 2e-2 tolerance"):
                    nc.vector.reciprocal(r_t, rtmp)
                rb_ps = ps_dr.tile([128, NW], F32, tag="dr", name="rb_ps")
                nc.tensor.matmul(rb_ps, exp_t, r_t, start=True, stop=True)
                # att = (num + CONST) * (1/denom)
                with nc.allow_low_precision(reason="bf16 within 2e-2 tolerance"):
                    nc.vector.scalar_tensor_tensor(
                        att_t[:, nb, :],
                        num_l[nb],
                        CONST,
                        rb_ps,
                        op0=ALU.add,
                        op1=ALU.mult,
                    )

        def emit_out(g):
            """mm2 + PSUM->SBUF copies + output DMA for group g."""
            st = state[g]
            att_t = st["att_t"]
            rot = COPY_ENG
            ev = 0
            for j in range(GB):
                for kc in range(KC):
                    o_sb = opool.tile([128, NCH, NW], BF16, tag="o", name="o_sb")
                    for nbp in range(NCH // 2):
                        o_ps = ps_o.tile([128, 2, NW], F32, tag="o", name="o_ps")
                        for h in range(2):
                            nb = 2 * nbp + h
                            if j < 3:
                                nc.tensor.matmul(
                                    o_ps[:, h, :],
                                    ftp_t[32 * j : 32 * j + HW, g, kc, :],
                                    att_t[32 * j : 32 * j + HW, nb, :],
                                    start=True,
                                    stop=True,
                                )
                            else:
                                nc.tensor.matmul(
                                    o_ps[:, h, :],
                                    ftp3_t[:, g, kc, :],
                                    att_t[:, nb, :],
                                    start=True,
                                    stop=True,
                                )
                        dst = o_sb[:, 2 * nbp : 2 * nbp + 2, :]
                        eng = rot[ev % 16]
                        ev += 1
                        with nc.allow_low_precision(
                            reason="bf16 within 2e-2 tolerance"
                        ):
                            if eng == "A":
                                nc.scalar.copy(dst, o_ps)
                            else:
                                nc.vector.tensor_copy(dst, o_ps)
                    nc.sync.dma_start(out=out_r[:, GB * g + j, kc], in_=o_sb)

        # iteration g: PE order is
        #   mm1(g, chunk0) | denom/bcast(g-1) | mm2(g-1) | mm1(g, chunks 1-3)
        # so mm2(g-1) (whose copies feed the output DMAs) starts as early as
        # possible while the denom matmuls still never stall the PE queue:
        # num(g-1) is computed on DVE at iteration start from last iteration's
        # relu/ln tiles.
        def iteration(g):
            if g + 1 < NG:
                issue_weights(g + 1)
            state[g] = {"abs_l": [], "relu_l": [], "ln_l": []}
            if g > 0:
                num_phase(g - 1)
            mm1_chunk(g, 0)
            if g > 0:
                tail(g - 1)
                emit_out(g - 1)
            for nb in range(1, NCH):
                mm1_chunk(g, nb)
            act_phases(g)

        for g in range(NG):
            iteration(g)
        num_phase(NG - 1)
        tail(NG - 1)
        emit_out(NG - 1)

    nc.compile()
    return nc


_NC_CACHE = {}


def _get_nc():
    if "nc" not in _NC_CACHE:
        _NC_CACHE["nc"] = build_nc()
    return _NC_CACHE["nc"]


def prep_features(features):
    """[B_FULL, C, H, W] f32 -> (fpad [128, KC, B_FULL, 32],
    ftpack [128, ngrp_total, KC, 128], ftpad3 [128, ngrp_total, KC, 128])
    all bf16.

    fpad[ci, kc, b, h] = f[b, kc*128+ci, h] (h < 16; zero-padded to 32).
    ftpack[32j+h, G, kc, ci] = f[4G+j, kc*128+ci, h]: fT at each slot's
    partition offset (mm2 stationary slices for slots 0-2).
    ftpad3 is slot 3's fT at partitions 96..111 with zeros elsewhere (mm2
    operand base partitions may only be 0/32/64, so slot 3 runs full-K).
    """
    f = np.asarray(features, np.float32).reshape(B_FULL, KC, 128, HW)
    fpad = np.zeros((B_FULL, KC, 128, 32), np.float32)
    fpad[..., :HW] = f
    fpad = np.ascontiguousarray(fpad.transpose(2, 1, 0, 3)).astype(NPBF16)

    ngrp = B_FULL // GB
    fg = f.reshape(ngrp, GB, KC, 128, HW)
    ftp = np.zeros((GB, 32, ngrp, KC, 128), np.float32)
    for j in range(GB):
        # [G, kc, ci, h] -> [h, G, kc, ci]
        ftp[j, :HW] = fg[:, j].transpose(3, 0, 1, 2)
    ftp3 = np.zeros_like(ftp)
    ftp3[3, :HW] = ftp[3, :HW]
    ftp = np.ascontiguousarray(ftp.reshape(128, ngrp, KC, 128)).astype(NPBF16)
    ftp3 = np.ascontiguousarray(ftp3.reshape(128, ngrp, KC, 128)).astype(NPBF16)
    return fpad, ftp, ftp3


def run(features, weights, trace=False, **kwargs):
    """Shard over 8 cores, run, gather. Returns (out, BassKernelResults)."""
    fpad, ftp, ftp3 = prep_features(features)
    weights = np.asarray(weights, np.float32).astype(NPBF16)
    aux = aux_inputs()
    nc = _get_nc()
    in_maps = []
    for i in range(N_CORES):
        sl = slice(i * B, (i + 1) * B)
        gsl = slice(i * NG, (i + 1) * NG)
        in_maps.append(
            {
                "fpad": np.ascontiguousarray(fpad[:, :, sl]),
                "ftpack": np.ascontiguousarray(ftp[:, gsl]),
                "ftpad3": np.ascontiguousarray(ftp3[:, gsl]),
                "weights": weights[sl],
                **aux,
            }
        )
    res = run_bass_kernel_spmd(
        nc, in_maps, core_ids=list(range(N_CORES)), trace=trace, **kwargs
    )
    out = np.concatenate([r["out"] for r in res.results], axis=0).astype(np.float32)
    return out, res


def kernel(features, weights):
    out, _ = run(features, weights)
    return out


# revision 51
# speedup vs baseline: 2.3288x; 1.1498x over previous
"""Attentional pooling layer on Trainium2 (Bass/Tile), 8-core batch-parallel.

Reference computation per batch b:
    scores[hw, n] = sum_c f[c, hw] * w[c, n]          (mm1)
    num           = softplus(scores)                  (relu/abs/Exp + quad poly)
    denom[n]      = sum_hw num[hw, n] + 16*CONST      (PE reduce, pad-row trick)
    att[hw, n]    = (num + CONST) / denom[n]          (DVE recip, PE bcast, DVE)
    out[c, n]     = sum_hw f[c, hw] * att[hw, n]      (mm2)

The kernel is DMA-bound (weights in + out out dominate), so both weights and
output travel as bf16 (tolerance is 2e-2; this lands ~4e-3), and all matmuls
take bf16 moving operands.  4 batches are packed per 128-partition group at
32-partition offsets 0/32/64/96 (PE tile_position); mm1 runs M=32 with
zero-padded feature columns so the 16 garbage rows per 32-block hold clean
zeros.  softplus(x) = max(x,0) + ln(1+exp(-|x|)); the ln(1+t) factor is a
minimax quadratic C1*t + C2*t^2 (max err 4.5e-3), so the only table-based
activation is Exp — a single table load for the whole kernel.  |x| is
2*relu(x) - x because an op may read PSUM only once; GPSIMD cannot touch PSUM
at all, so Pool evaluates the (all-SBUF) polynomial and num sum while the
PSUM->SBUF output copies rotate over ACT/DVE.  The denominator's +16*CONST
rides inside the bd reduction matmul: each block's 16 zero-pad rows carry
weight WPAD = CONST/poly(1) instead of 0.

The per-group tail (reduce/broadcast/att/mm2) is software-pipelined one group
behind mm1 so the in-order PE queue never waits on the softplus chain:
PE program is mm1(g, chunk0) | denom+bcast(g-1) | mm2(g-1) | mm1(g, 1-3).

32 batches per core = 8 groups of 4, no ragged tail.
"""

import numpy as np
import ml_dtypes
from contextlib import ExitStack

import concourse.bass as bass
import concourse.bacc as bacc
import concourse.tile as tile
from concourse import mybir
from concourse.bass_utils import run_bass_kernel_spmd

F32 = mybir.dt.float32
BF16 = mybir.dt.bfloat16
NPBF16 = ml_dtypes.bfloat16
AF = mybir.ActivationFunctionType
ALU = mybir.AluOpType

N_CORES = 8
B_FULL, C, H, W, N = 256, 256, 4, 4, 2048
HW = H * W                  # 16
B = B_FULL // N_CORES       # 32 batches per core
KC = C // 128               # 2 contraction chunks of 128
GB = 4                      # batches per partition group (offsets 0/32/64/96)
NG = B // GB                # 8 groups
NCH = 4                     # n chunks per group chain
NW = N // NCH               # 512 (one PSUM bank)
CONST = 1e-4

# ln(1+t) ~ C1*t + C2*t^2 on [0,1], minimax through origin (max err 4.5e-3)
C1, C2 = 0.94, -0.251
# poly(1) evaluated exactly as the bf16 pipeline does for the zero-pad rows
# (t = exp(0) = 1): v1 = bf16(C2 + C1); c = bf16(v1); num_pad = bf16(c)
POLY1 = float(
    np.float32(NPBF16(np.float32(NPBF16(np.float32(C1 + C2)))))
)
WPAD = CONST / POLY1


def aux_inputs():
    # bd[k, m]: per batch-slot m, weight 1 for its 16 real hw rows and WPAD
    # for its 16 zero-pad rows (whose num is exactly poly(1)), so the bd
    # reduction yields sum_hw softplus + 16*CONST directly
    bd = np.zeros((128, GB), NPBF16)
    for k in range(128):
        bd[k, k // 32] = 1.0 if k % 32 < HW else WPAD
    # exp4[m, p] = 1 iff partition p belongs to batch-slot m's 32-block
    exp4 = np.zeros((GB, 128), NPBF16)
    for p in range(128):
        exp4[p // 32, p] = 1.0
    return {"bd": bd, "exp4": exp4}


def build_nc(debug=False):
    nc = bacc.Bacc(None, target_bir_lowering=False, debug=debug)
    feat = nc.dram_tensor("fpad", [128, KC, B, 32], BF16, kind="ExternalInput")
    ftp = nc.dram_tensor("ftpack", [128, NG, KC, 128], BF16, kind="ExternalInput")
    ftp3 = nc.dram_tensor("ftpad3", [128, NG, KC, 128], BF16, kind="ExternalInput")
    wts = nc.dram_tensor("weights", [B, C, N], BF16, kind="ExternalInput")
    out = nc.dram_tensor("out", [B, C, N], BF16, kind="ExternalOutput")
    bd_d = nc.dram_tensor("bd", [128, GB], BF16, kind="ExternalInput")
    exp_d = nc.dram_tensor("exp4", [GB, 128], BF16, kind="ExternalInput")

    # [ci, b, kc, nch, nw] views of the DRAM tensors
    wts_r = wts.ap().rearrange("b (kc ci) (nch nw) -> ci b kc nch nw", kc=KC, nch=NCH)
    out_r = out.ap().rearrange("b (kc ci) (nch nw) -> ci b kc nch nw", kc=KC, nch=NCH)

    with tile.TileContext(nc) as tc, ExitStack() as ctx:
        singles = ctx.enter_context(tc.tile_pool(name="singles", bufs=1))
        wpool = ctx.enter_context(tc.tile_pool(name="w", bufs=12))
        opool = ctx.enter_context(tc.tile_pool(name="o", bufs=6))
        numpool = ctx.enter_context(tc.tile_pool(name="num", bufs=6))
        attpool = ctx.enter_context(tc.tile_pool(name="att", bufs=3))
        smallpool = ctx.enter_context(tc.tile_pool(name="small", bufs=3))
        ps_sc = ctx.enter_context(tc.tile_pool(name="ps_sc", bufs=2, space="PSUM"))
        ps_dr = ctx.enter_context(tc.tile_pool(name="ps_dr", bufs=2, space="PSUM"))
        ps_o = ctx.enter_context(tc.tile_pool(name="ps_o", bufs=2, space="PSUM"))

        w_t = {}

        def issue_weights(g):
            for b in range(GB * g, GB * (g + 1)):
                w_t[b] = wpool.tile([128, KC, NCH, NW], BF16, tag="w", name="w_t")
                nc.sync.dma_start(out=w_t[b], in_=wts_r[:, b])

        # features first (mm1's stationary), then the first weight group, then
        # the small/later-needed tensors — keeps the DMA queue dense at start
        # while letting mm1 begin as early as possible
        f_t = singles.tile([128, KC, B, 32], BF16, name="f_t")
        nc.sync.dma_start(out=f_t, in_=feat.ap())
        bd_t = singles.tile([128, GB], BF16, name="bd_t")
        nc.sync.dma_start(out=bd_t, in_=bd_d.ap())
        issue_weights(0)
        exp_t = singles.tile([GB, 128], BF16, name="exp_t")
        nc.sync.dma_start(out=exp_t, in_=exp_d.ap())
        # fT[hw, c] per batch at its slot's partition offset (mm2 stationary);
        # slot 3 additionally as a full-K tile with zeros outside rows 96..111
        ftp_t = singles.tile([128, NG, KC, 128], BF16, name="ftp_t")
        nc.sync.dma_start(out=ftp_t, in_=ftp.ap())
        ftp3_t = singles.tile([128, NG, KC, 128], BF16, name="ftp3_t")
        nc.sync.dma_start(out=ftp3_t, in_=ftp3.ap())

        # copy-engine rotation per [128, 2*NW] pair-copy: GPSIMD cannot read
        # PSUM on real TRN2, so the PSUM->SBUF output copies are split across
        # ACT (x10) and DVE (x6) only; Pool evaluates the softplus polynomial
        COPY_ENG = ["A", "D", "A", "A", "D", "A", "D", "A",
                    "A", "D", "A", "D", "A", "A", "D", "A"]
        state = {}      # g -> dict with num_l, att_t

        def mm1_chunk(g, nb):
            """mm1 for chunk nb of group g + the full softplus chain:
            relu (ACT), |x| = 2*relu - x (DVE), t = exp(-|x|) (ACT),
            c = t*(C1 + C2*t) and num = relu + c (Pool, all-SBUF)."""
            st = state[g]
            sc_ps = ps_sc.tile([128, NW], F32, tag="sc", name="sc_ps")
            for j in range(GB):
                for kc in range(KC):
                    nc.tensor.matmul(
                        sc_ps[32 * j : 32 * j + 32, :],
                        f_t[:, kc, GB * g + j, :],
                        w_t[GB * g + j][:, kc, nb, :],
                        start=(kc == 0),
                        stop=(kc == KC - 1),
                        tile_position=(0, 32 * j),
                    )
            t_relu = numpool.tile([128, NW], BF16, tag="trelu", bufs=9, name="t_relu")
            t_abs = numpool.tile([128, NW], BF16, tag="tabs", bufs=6, name="t_abs")
            t_exp = numpool.tile([128, NW], BF16, tag="texp", bufs=6, name="t_exp")
            t_v1 = numpool.tile([128, NW], BF16, tag="tv1", bufs=4, name="t_v1")
            t_c = numpool.tile([128, NW], BF16, tag="tc", bufs=4, name="t_c")
            num_t = numpool.tile([128, NW], BF16, tag="num", bufs=6, name="num_t")
            with nc.allow_low_precision(reason="bf16 within 2e-2 tolerance"):
                nc.scalar.activation(t_relu, sc_ps, AF.Relu)
                nc.vector.scalar_tensor_tensor(
                    t_abs, t_relu, 2.0, sc_ps, op0=ALU.mult, op1=ALU.subtract
                )
                nc.scalar.activation(t_exp, t_abs, AF.Exp, scale=-1.0)
                nc.gpsimd.tensor_scalar(
                    t_v1, t_exp, C2, C1, op0=ALU.mult, op1=ALU.add
                )
                nc.gpsimd.tensor_mul(t_c, t_v1, t_exp)
                nc.gpsimd.tensor_add(num_t, t_relu, t_c)
            st["num_l"].append(num_t)

        def tail(g):
            """denom/recip/bcast/att for group g (emitted after mm1(g+1,0)
            so the in-order PE queue has num(g) ready)."""
            st = state[g]
            att_t = st["att_t"] = attpool.tile([128, NCH, NW], BF16, name="att_t")
            num_l = st["num_l"]
            for nb in range(NCH):
                d_ps = ps_dr.tile([GB, NW], F32, tag="dr", name="d_ps")
                nc.tensor.matmul(d_ps, bd_t, num_l[nb], start=True, stop=True)
                r_t = smallpool.tile([GB, NW], BF16, tag="rb", name="r_t")
                with nc.allow_low_precision(reason="bf16 within 2e-2 tolerance"):
                    nc.vector.reciprocal(r_t, d_ps)
                rb_ps = ps_dr.tile([128, NW], F32, tag="dr", name="rb_ps")
                nc.tensor.matmul(rb_ps, exp_t, r_t, start=True, stop=True)
                # att = (num + CONST) * (1/denom)
                with nc.allow_low_precision(reason="bf16 within 2e-2 tolerance"):
                    nc.vector.scalar_tensor_tensor(
                        att_t[:, nb, :],
                        num_l[nb],
                        CONST,
                        rb_ps,
                        op0=ALU.add,
                        op1=ALU.mult,
                    )

        def emit_out(g):
            """mm2 + PSUM->SBUF copies + output DMA for group g."""
            st = state[g]
            att_t = st["att_t"]
            ev = 0
            for j in range(GB):
                for kc in range(KC):
                    o_sb = opool.tile([128, NCH, NW], BF16, tag="o", name="o_sb")
                    for nbp in range(NCH // 2):
                        o_ps = ps_o.tile([128, 2, NW], F32, tag="o", name="o_ps")
                        for h in range(2):
                            nb = 2 * nbp + h
                            if j < 3:
                                nc.tensor.matmul(
                                    o_ps[:, h, :],
                                    ftp_t[32 * j : 32 * j + HW, g, kc, :],
                                    att_t[32 * j : 32 * j + HW, nb, :],
                                    start=True,
                                    stop=True,
                                )
                            else:
                                nc.tensor.matmul(
                                    o_ps[:, h, :],
                                    ftp3_t[:, g, kc, :],
                                    att_t[:, nb, :],
                                    start=True,
                                    stop=True,
                                )
                        dst = o_sb[:, 2 * nbp : 2 * nbp + 2, :]
                        eng = COPY_ENG[ev % 16]
                        ev += 1
                        with nc.allow_low_precision(
                            reason="bf16 within 2e-2 tolerance"
                        ):
                            if eng == "A":
                                nc.scalar.copy(dst, o_ps)
                            else:
                                nc.vector.tensor_copy(dst, o_ps)
                    nc.sync.dma_start(out=out_r[:, GB * g + j, kc], in_=o_sb)

        # iteration g: PE order is
        #   mm1(g, chunk0) | denom/bcast(g-1) | mm2(g-1) | mm1(g, chunks 1-3)
        # so mm2(g-1) (whose copies feed the output DMAs) starts as early as
        # possible while the denom matmuls still never stall the PE queue.
        def iteration(g):
            if g + 1 < NG:
                issue_weights(g + 1)
            state[g] = {"num_l": []}
            mm1_chunk(g, 0)
            if g > 0:
                tail(g - 1)
                emit_out(g - 1)
            for nb in range(1, NCH):
                mm1_chunk(g, nb)

        for g in range(NG):
            iteration(g)
        tail(NG - 1)
        emit_out(NG - 1)

    nc.compile()
    return nc


_NC_CACHE = {}


def _get_nc():
    if "nc" not in _NC_CACHE:
        _NC_CACHE["nc"] = build_nc()
    return _NC_CACHE["nc"]


def prep_features(features):
    """[B_FULL, C, H, W] f32 -> (fpad [128, KC, B_FULL, 32],
    ftpack [128, ngrp_total, KC, 128], ftpad3 [128, ngrp_total, KC, 128])
    all bf16.

    fpad[ci, kc, b, h] = f[b, kc*128+ci, h] (h < 16; zero-padded to 32).
    ftpack[32j+h, G, kc, ci] = f[4G+j, kc*128+ci, h]: fT at each slot's
    partition offset (mm2 stationary slices for slots 0-2).
    ftpad3 is slot 3's fT at partitions 96..111 with zeros elsewhere (mm2
    operand base partitions may only be 0/32/64, so slot 3 runs full-K).
    """
    f = np.asarray(features, np.float32).reshape(B_FULL, KC, 128, HW)
    fpad = np.zeros((B_FULL, KC, 128, 32), np.float32)
    fpad[..., :HW] = f
    fpad = np.ascontiguousarray(fpad.transpose(2, 1, 0, 3)).astype(NPBF16)

    ngrp = B_FULL // GB
    fg = f.reshape(ngrp, GB, KC, 128, HW)
    ftp = np.zeros((GB, 32, ngrp, KC, 128), np.float32)
    for j in range(GB):
        # [G, kc, ci, h] -> [h, G, kc, ci]
        ftp[j, :HW] = fg[:, j].transpose(3, 0, 1, 2)
    ftp3 = np.zeros_like(ftp)
    ftp3[3, :HW] = ftp[3, :HW]
    ftp = np.ascontiguousarray(ftp.reshape(128, ngrp, KC, 128)).astype(NPBF16)
    ftp3 = np.ascontiguousarray(ftp3.reshape(128, ngrp, KC, 128)).astype(NPBF16)
    return fpad, ftp, ftp3


def run(features, weights, trace=False, **kwargs):
    """Shard over 8 cores, run, gather. Returns (out, BassKernelResults)."""
    fpad, ftp, ftp3 = prep_features(features)
    weights = np.asarray(weights, np.float32).astype(NPBF16)
    aux = aux_inputs()
    nc = _get_nc()
    in_maps = []
    for i in range(N_CORES):
        sl = slice(i * B, (i + 1) * B)
        gsl = slice(i * NG, (i + 1) * NG)
        in_maps.append(
            {
                "fpad": np.ascontiguousarray(fpad[:, :, sl]),
                "ftpack": np.ascontiguousarray(ftp[:, gsl]),
                "ftpad3": np.ascontiguousarray(ftp3[:, gsl]),
                "weights": weights[sl],
                **aux,
            }
        )
    res = run_bass_kernel_spmd(
        nc, in_maps, core_ids=list(range(N_CORES)), trace=trace, **kwargs
    )
    out = np.concatenate([r["out"] for r in res.results], axis=0).astype(np.float32)
    return out, res


def kernel(features, weights):
    out, _ = run(features, weights)
    return out


# revision 59
# speedup vs baseline: 2.3413x; 1.0054x over previous
"""Attentional pooling layer on Trainium2 (Bass/Tile), 8-core batch-parallel.

Reference computation per batch b:
    scores[hw, n] = sum_c f[c, hw] * w[c, n]          (mm1)
    num           = softplus(scores)                  (relu/abs/Exp + quad poly)
    denom[n]      = sum_hw num[hw, n] + 16*CONST      (PE reduce, pad-row trick)
    att[hw, n]    = (num + CONST) / denom[n]          (DVE recip, PE bcast, DVE)
    out[c, n]     = sum_hw f[c, hw] * att[hw, n]      (mm2)

The kernel is DMA-bound (weights in + out out dominate), so both weights and
output travel as bf16 (tolerance is 2e-2; this lands ~4e-3), and all matmuls
take bf16 moving operands.  4 batches are packed per 128-partition group at
32-partition offsets 0/32/64/96 (PE tile_position); mm1 runs M=32 with
zero-padded feature columns so the 16 garbage rows per 32-block hold clean
zeros.  softplus(x) = max(x,0) + ln(1+exp(-|x|)); the ln(1+t) factor is a
minimax quadratic C1*t + C2*t^2 (max err 4.5e-3), so the only table-based
activation is Exp — a single table load for the whole kernel.  |x| is
2*relu(x) - x because an op may read PSUM only once; GPSIMD cannot touch PSUM
at all, so Pool evaluates the (all-SBUF) polynomial and num sum while the
PSUM->SBUF output copies rotate over ACT/DVE.  The denominator's +16*CONST
rides inside the bd reduction matmul: each block's 16 zero-pad rows carry
weight WPAD = CONST/poly(1) instead of 0.

The per-group tail (reduce/broadcast/att/mm2) is software-pipelined one group
behind mm1 so the in-order PE queue never waits on the softplus chain:
PE program is mm1(g, chunk0) | denom+bcast(g-1) | mm2(g-1) | mm1(g, 1-3).

32 batches per core = 8 groups of 4, no ragged tail.
"""

import numpy as np
import ml_dtypes
from contextlib import ExitStack

import concourse.bass as bass
import concourse.bacc as bacc
import concourse.tile as tile
from concourse import mybir
from concourse.bass_utils import run_bass_kernel_spmd

F32 = mybir.dt.float32
BF16 = mybir.dt.bfloat16
NPBF16 = ml_dtypes.bfloat16
AF = mybir.ActivationFunctionType
ALU = mybir.AluOpType

N_CORES = 8
B_FULL, C, H, W, N = 256, 256, 4, 4, 2048
HW = H * W                  # 16
B = B_FULL // N_CORES       # 32 batches per core
KC = C // 128               # 2 contraction chunks of 128
GB = 4                      # batches per partition group (offsets 0/32/64/96)
NG = B // GB                # 8 groups
NCH = 4                     # n chunks per group chain
NW = N // NCH               # 512 (one PSUM bank)
CONST = 1e-4

# ln(1+t) ~ C1*t + C2*t^2 on [0,1], minimax through origin (max err 4.5e-3)
C1, C2 = 0.94, -0.251
# poly(1) evaluated exactly as the bf16 pipeline does for the zero-pad rows
# (t = exp(0) = 1): v1 = bf16(C2 + C1); c = bf16(v1); num_pad = bf16(c)
POLY1 = float(
    np.float32(NPBF16(np.float32(NPBF16(np.float32(C1 + C2)))))
)
WPAD = CONST / POLY1


def aux_inputs():
    # bd[k, m]: per batch-slot m, weight 1 for its 16 real hw rows and WPAD
    # for its 16 zero-pad rows (whose num is exactly poly(1)), so the bd
    # reduction yields sum_hw softplus + 16*CONST directly
    bd = np.zeros((128, GB), NPBF16)
    for k in range(128):
        bd[k, k // 32] = 1.0 if k % 32 < HW else WPAD
    # exp4[m, p] = 1 iff partition p belongs to batch-slot m's 32-block
    exp4 = np.zeros((GB, 128), NPBF16)
    for p in range(128):
        exp4[p // 32, p] = 1.0
    return {"bd": bd, "exp4": exp4}


def build_nc(debug=False):
    nc = bacc.Bacc(None, target_bir_lowering=False, debug=debug)
    feat = nc.dram_tensor("fpad", [128, KC, B, 32], BF16, kind="ExternalInput")
    ftp = nc.dram_tensor("ftpack", [128, NG, KC, 128], BF16, kind="ExternalInput")
    ftp3 = nc.dram_tensor("ftpad3", [128, NG, KC, 128], BF16, kind="ExternalInput")
    wts = nc.dram_tensor("weights", [B, C, N], BF16, kind="ExternalInput")
    out = nc.dram_tensor("out", [B, C, N], BF16, kind="ExternalOutput")
    bd_d = nc.dram_tensor("bd", [128, GB], BF16, kind="ExternalInput")
    exp_d = nc.dram_tensor("exp4", [GB, 128], BF16, kind="ExternalInput")

    # [ci, b, kc, nch, nw] views of the DRAM tensors
    wts_r = wts.ap().rearrange("b (kc ci) (nch nw) -> ci b kc nch nw", kc=KC, nch=NCH)
    out_r = out.ap().rearrange("b (kc ci) (nch nw) -> ci b kc nch nw", kc=KC, nch=NCH)

    with tile.TileContext(nc) as tc, ExitStack() as ctx:
        singles = ctx.enter_context(tc.tile_pool(name="singles", bufs=1))
        wpool = ctx.enter_context(tc.tile_pool(name="w", bufs=12))
        opool = ctx.enter_context(tc.tile_pool(name="o", bufs=6))
        numpool = ctx.enter_context(tc.tile_pool(name="num", bufs=6))
        attpool = ctx.enter_context(tc.tile_pool(name="att", bufs=3))
        smallpool = ctx.enter_context(tc.tile_pool(name="small", bufs=3))
        ps_sc = ctx.enter_context(tc.tile_pool(name="ps_sc", bufs=2, space="PSUM"))
        ps_dr = ctx.enter_context(tc.tile_pool(name="ps_dr", bufs=2, space="PSUM"))
        ps_o = ctx.enter_context(tc.tile_pool(name="ps_o", bufs=2, space="PSUM"))

        w_t = {}

        def issue_weights(g):
            for b in range(GB * g, GB * (g + 1)):
                w_t[b] = wpool.tile([128, KC, NCH, NW], BF16, tag="w", name="w_t")
                nc.sync.dma_start(out=w_t[b], in_=wts_r[:, b])

        # features first (mm1's stationary), then the first weight group, then
        # the small/later-needed tensors — keeps the DMA queue dense at start
        # while letting mm1 begin as early as possible
        f_t = singles.tile([128, KC, B, 32], BF16, name="f_t")
        nc.sync.dma_start(out=f_t, in_=feat.ap())
        bd_t = singles.tile([128, GB], BF16, name="bd_t")
        nc.sync.dma_start(out=bd_t, in_=bd_d.ap())
        issue_weights(0)
        exp_t = singles.tile([GB, 128], BF16, name="exp_t")
        nc.sync.dma_start(out=exp_t, in_=exp_d.ap())
        # fT[hw, c] per batch at its slot's partition offset (mm2 stationary);
        # slot 3 additionally as a full-K tile with zeros outside rows 96..111
        ftp_t = singles.tile([128, NG, KC, 128], BF16, name="ftp_t")
        nc.sync.dma_start(out=ftp_t, in_=ftp.ap())
        ftp3_t = singles.tile([128, NG, KC, 128], BF16, name="ftp3_t")
        nc.sync.dma_start(out=ftp3_t, in_=ftp3.ap())

        # copy-engine rotation per [128, 2*NW] pair-copy: GPSIMD cannot read
        # PSUM on real TRN2, so the PSUM->SBUF output copies are split across
        # ACT (x10) and DVE (x6) only; Pool evaluates the softplus polynomial
        COPY_ENG = ["A", "D", "A", "A", "D", "A", "D", "A",
                    "A", "D", "A", "D", "A", "A", "D", "A"]
        # in the drain (last group) ACT has no next group's relu/exp to run,
        # so split copies evenly to keep piece production above the DMA rate
        DRAIN_ENG = ["A", "D"] * 8
        state = {}      # g -> dict with num_l, att_t

        def mm1_chunk(g, nb):
            """mm1 for chunk nb of group g + the full softplus chain:
            relu (ACT), |x| = 2*relu - x (DVE), t = exp(-|x|) (ACT),
            c = t*(C1 + C2*t) and num = relu + c (Pool, all-SBUF)."""
            st = state[g]
            sc_ps = ps_sc.tile([128, NW], F32, tag="sc", name="sc_ps")
            for j in range(GB):
                for kc in range(KC):
                    nc.tensor.matmul(
                        sc_ps[32 * j : 32 * j + 32, :],
                        f_t[:, kc, GB * g + j, :],
                        w_t[GB * g + j][:, kc, nb, :],
                        start=(kc == 0),
                        stop=(kc == KC - 1),
                        tile_position=(0, 32 * j),
                    )
            t_relu = numpool.tile([128, NW], BF16, tag="trelu", bufs=9, name="t_relu")
            t_abs = numpool.tile([128, NW], BF16, tag="tabs", bufs=6, name="t_abs")
            t_exp = numpool.tile([128, NW], BF16, tag="texp", bufs=6, name="t_exp")
            t_v1 = numpool.tile([128, NW], BF16, tag="tv1", bufs=4, name="t_v1")
            t_c = numpool.tile([128, NW], BF16, tag="tc", bufs=4, name="t_c")
            num_t = numpool.tile([128, NW], BF16, tag="num", bufs=6, name="num_t")
            with nc.allow_low_precision(reason="bf16 within 2e-2 tolerance"):
                nc.scalar.activation(t_relu, sc_ps, AF.Relu)
                nc.vector.scalar_tensor_tensor(
                    t_abs, t_relu, 2.0, sc_ps, op0=ALU.mult, op1=ALU.subtract
                )
                nc.scalar.activation(t_exp, t_abs, AF.Exp, scale=-1.0)
                nc.gpsimd.tensor_scalar(
                    t_v1, t_exp, C2, C1, op0=ALU.mult, op1=ALU.add
                )
                nc.gpsimd.tensor_mul(t_c, t_v1, t_exp)
                nc.gpsimd.tensor_add(num_t, t_relu, t_c)
            st["num_l"].append(num_t)

        def tail(g):
            """denom/recip/bcast/att for group g (emitted after mm1(g+1,0)
            so the in-order PE queue has num(g) ready)."""
            st = state[g]
            att_t = st["att_t"] = attpool.tile([128, NCH, NW], BF16, name="att_t")
            num_l = st["num_l"]
            for nb in range(NCH):
                d_ps = ps_dr.tile([GB, NW], F32, tag="dr", name="d_ps")
                nc.tensor.matmul(d_ps, bd_t, num_l[nb], start=True, stop=True)
                r_t = smallpool.tile([GB, NW], BF16, tag="rb", name="r_t")
                with nc.allow_low_precision(reason="bf16 within 2e-2 tolerance"):
                    nc.vector.reciprocal(r_t, d_ps)
                rb_ps = ps_dr.tile([128, NW], F32, tag="dr", name="rb_ps")
                nc.tensor.matmul(rb_ps, exp_t, r_t, start=True, stop=True)
                # att = (num + CONST) * (1/denom)
                with nc.allow_low_precision(reason="bf16 within 2e-2 tolerance"):
                    nc.vector.scalar_tensor_tensor(
                        att_t[:, nb, :],
                        num_l[nb],
                        CONST,
                        rb_ps,
                        op0=ALU.add,
                        op1=ALU.mult,
                    )

        def emit_out(g):
            """mm2 + PSUM->SBUF copies + output DMA for group g."""
            st = state[g]
            att_t = st["att_t"]
            rot = COPY_ENG
            # in the drain there are no weight transfers left to hide the
            # mm2->copy->DMA latency, so ship each pair as its own DMA
            split_dma = g >= NG - 2
            ev = 0
            for j in range(GB):
                for kc in range(KC):
                    o_sb = opool.tile([128, NCH, NW], BF16, tag="o", name="o_sb")
                    for nbp in range(NCH // 2):
                        o_ps = ps_o.tile([128, 2, NW], F32, tag="o", name="o_ps")
                        for h in range(2):
                            nb = 2 * nbp + h
                            if j < 3:
                                nc.tensor.matmul(
                                    o_ps[:, h, :],
                                    ftp_t[32 * j : 32 * j + HW, g, kc, :],
                                    att_t[32 * j : 32 * j + HW, nb, :],
                                    start=True,
                                    stop=True,
                                )
                            else:
                                nc.tensor.matmul(
                                    o_ps[:, h, :],
                                    ftp3_t[:, g, kc, :],
                                    att_t[:, nb, :],
                                    start=True,
                                    stop=True,
                                )
                        dst = o_sb[:, 2 * nbp : 2 * nbp + 2, :]
                        eng = rot[ev % 16]
                        ev += 1
                        with nc.allow_low_precision(
                            reason="bf16 within 2e-2 tolerance"
                        ):
                            if eng == "A":
                                nc.scalar.copy(dst, o_ps)
                            else:
                                nc.vector.tensor_copy(dst, o_ps)
                        if split_dma:
                            nc.sync.dma_start(
                                out=out_r[
                                    :, GB * g + j, kc, 2 * nbp : 2 * nbp + 2
                                ],
                                in_=dst,
                            )
                    if not split_dma:
                        nc.sync.dma_start(out=out_r[:, GB * g + j, kc], in_=o_sb)

        # iteration g: PE order is
        #   mm1(g, chunk0) | denom/bcast(g-1) | mm2(g-1) | mm1(g, chunks 1-3)
        # so mm2(g-1) (whose copies feed the output DMAs) starts as early as
        # possible while the denom matmuls still never stall the PE queue.
        def iteration(g):
            if g + 1 < NG:
                issue_weights(g + 1)
            state[g] = {"num_l": []}
            mm1_chunk(g, 0)
            if g > 0:
                tail(g - 1)
                emit_out(g - 1)
            for nb in range(1, NCH):
                mm1_chunk(g, nb)

        for g in range(NG):
            iteration(g)
        tail(NG - 1)

        # epilogue: nothing overlaps the last group's output, so emit all
        # first-half pairs (att chunks 0-1) before the second half and ship
        # each pair as its own DMA the moment its copy lands
        g = NG - 1
        att_t = state[g]["att_t"]
        ev = 0
        for nbp in range(NCH // 2):
            for j in range(GB):
                for kc in range(KC):
                    o_ps = ps_o.tile([128, 2, NW], F32, tag="o", name="o_ps")
                    for h in range(2):
                        nb = 2 * nbp + h
                        if j < 3:
                            nc.tensor.matmul(
                                o_ps[:, h, :],
                                ftp_t[32 * j : 32 * j + HW, g, kc, :],
                                att_t[32 * j : 32 * j + HW, nb, :],
                                start=True,
                                stop=True,
                            )
                        else:
                            nc.tensor.matmul(
                                o_ps[:, h, :],
                                ftp3_t[:, g, kc, :],
                                att_t[:, nb, :],
                                start=True,
                                stop=True,
                            )
                    o2 = opool.tile([128, 2, NW], BF16, tag="olast", bufs=6,
                                    name="o2")
                    eng = ["A", "D"][ev % 2]
                    ev += 1
                    with nc.allow_low_precision(
                        reason="bf16 within 2e-2 tolerance"
                    ):
                        if eng == "A":
                            nc.scalar.copy(o2, o_ps)
                        else:
                            nc.vector.tensor_copy(o2, o_ps)
                    nc.sync.dma_start(
                        out=out_r[:, GB * g + j, kc, 2 * nbp : 2 * nbp + 2],
                        in_=o2,
                    )

    nc.compile()
    return nc


_NC_CACHE = {}


def _get_nc():
    if "nc" not in _NC_CACHE:
        _NC_CACHE["nc"] = build_nc()
    return _NC_CACHE["nc"]


def prep_features(features):
    """[B_FULL, C, H, W] f32 -> (fpad [128, KC, B_FULL, 32],
    ftpack [128, ngrp_total, KC, 128], ftpad3 [128, ngrp_total, KC, 128])
    all bf16.

    fpad[ci, kc, b, h] = f[b, kc*128+ci, h] (h < 16; zero-padded to 32).
    ftpack[32j+h, G, kc, ci] = f[4G+j, kc*128+ci, h]: fT at each slot's
    partition offset (mm2 stationary slices for slots 0-2).
    ftpad3 is slot 3's fT at partitions 96..111 with zeros elsewhere (mm2
    operand base partitions may only be 0/32/64, so slot 3 runs full-K).
    """
    f = np.asarray(features, np.float32).reshape(B_FULL, KC, 128, HW)
    fpad = np.zeros((B_FULL, KC, 128, 32), np.float32)
    fpad[..., :HW] = f
    fpad = np.ascontiguousarray(fpad.transpose(2, 1, 0, 3)).astype(NPBF16)

    ngrp = B_FULL // GB
    fg = f.reshape(ngrp, GB, KC, 128, HW)
    ftp = np.zeros((GB, 32, ngrp, KC, 128), np.float32)
    for j in range(GB):
        # [G, kc, ci, h] -> [h, G, kc, ci]
        ftp[j, :HW] = fg[:, j].transpose(3, 0, 1, 2)
    ftp3 = np.zeros_like(ftp)
    ftp3[3, :HW] = ftp[3, :HW]
    ftp = np.ascontiguousarray(ftp.reshape(128, ngrp, KC, 128)).astype(NPBF16)
    ftp3 = np.ascontiguousarray(ftp3.reshape(128, ngrp, KC, 128)).astype(NPBF16)
    return fpad, ftp, ftp3


def run(features, weights, trace=False, **kwargs):
    """Shard over 8 cores, run, gather. Returns (out, BassKernelResults)."""
    fpad, ftp, ftp3 = prep_features(features)
    weights = np.asarray(weights, np.float32).astype(NPBF16)
    aux = aux_inputs()
    nc = _get_nc()
    in_maps = []
    for i in range(N_CORES):
        sl = slice(i * B, (i + 1) * B)
        gsl = slice(i * NG, (i + 1) * NG)
        in_maps.append(
            {
                "fpad": np.ascontiguousarray(fpad[:, :, sl]),
                "ftpack": np.ascontiguousarray(ftp[:, gsl]),
                "ftpad3": np.ascontiguousarray(ftp3[:, gsl]),
                "weights": weights[sl],
                **aux,
            }
        )
    res = run_bass_kernel_spmd(
        nc, in_maps, core_ids=list(range(N_CORES)), trace=trace, **kwargs
    )
    out = np.concatenate([r["out"] for r in res.results], axis=0).astype(np.float32)
    return out, res


def kernel(features, weights):
    out, _ = run(features, weights)
    return out
